# revision 29
# baseline (speedup 1.0000x reference)
"""Multi-head attention (B=4, N=2048, C=1024, H=16, D=64) on 8 TRN2 cores.

Sharding: core c -> batch b = c%4, head-group g = c//4 (local heads 0..7 are
global heads 8g..8g+7).  Each core computes its head group's contribution to
the output projection for its batch; host sums core b + core b+4 and adds
const_row = qkv_b[2048:] @ proj_w + proj_b (V-bias folds exactly through the
row-normalized attention: attn @ (1*bv^T) = 1*bv^T).

Implementation: all matmul operands bf16 (fp32 PSUM accumulation), which
enables fast FWL weight loads (fp32r matmuls must self-load weights on this
toolchain and mixing 16/32-bit operands is rejected).  x and the weights are
resident in SBUF (full-row DMAs).  Prefix computes K, V, then Q for all
tokens; phase 2 is ONE continuous software pipeline over (query-block, head
-pair, key-group) steps - PV lags scores by one step across all block
boundaries so the PE never sees a refill bubble (keeps HAM at K=8/8) while
ACT (exp, ~2.3us per step) is the steady-state bottleneck at ~94% duty.
Softmax normalization: denominator row 64 of oaug -> SBUF staging copy
(frees the PSUM bank fast) -> DRAM bounce reshaped to [64,8] so the DVE
reciprocal runs wide (250ns, not 3.3us single-lane) -> broadcast read ->
deferred multiply (never blocks the strict-FIFO DVE queue on a DMA wait).
The projection of block i is interleaved into block i+1's steps; the final
drain emits dummy matmuls to keep the PE warm through the last normalize
round-trip.  gpsimd partition_broadcast and reciprocal_approx_fast are
numerically broken on this hardware - do not reintroduce them.
"""

import sys

sys.path.insert(0, "/opt/trn_rl_repo")

from contextlib import ExitStack

import ml_dtypes
import numpy as np

from concourse import bacc, mybir, tile
from concourse.bass_utils import run_bass_kernel_spmd

F32 = mybir.dt.float32
F32R = mybir.dt.float32r
BF16 = mybir.dt.bfloat16
EXP = mybir.ActivationFunctionType.Exp
ADD = mybir.AluOpType.add
MULT = mybir.AluOpType.mult

B, N, C, H, D = 4, 2048, 1024, 16, 64
SCALE = 0.125
TB = 512  # token block for phase A / Q pass


def _round_fp32r(a: np.ndarray) -> np.ndarray:
    b = np.ascontiguousarray(a, dtype=np.float32).view(np.uint32).astype(np.uint64)
    lsb = (b >> np.uint64(12)) & np.uint64(1)
    b = (b + np.uint64(0x7FF) + lsb) & np.uint64(0xFFFFF000)
    return b.astype(np.uint32).view(np.float32)


def _bf16(a: np.ndarray) -> np.ndarray:
    return np.ascontiguousarray(a, dtype=np.float32).astype(ml_dtypes.bfloat16)


class _QEmitter:
    """Emits the Q projection for one 512-token block in small steps so the
    matmuls can be sprinkled into phase-2 tg loops."""

    def __init__(self, nc, ps, w_sb, qb_sb, Q_T, slabQ, nb):
        self.nc, self.ps = nc, ps
        self.w_sb, self.qb_sb, self.Q_T, self.slabQ, self.nb = (
            w_sb, qb_sb, Q_T, slabQ, nb)
        self.steps = [(pr, j) for pr in range(4) for j in range(8)]
        self.idx = 0
        self.acc = None

    def done(self):
        return self.idx >= len(self.steps)

    def emit(self, k):
        nc = self.nc
        while k > 0 and not self.done():
            pr, j = self.steps[self.idx]
            if j == 0:
                self.acc = self.ps.tile([128, TB], F32, tag="qacc", bufs=1)
            nc.tensor.matmul(self.acc[:],
                             self.w_sb[:, j, pr * 128:(pr + 1) * 128],
                             self.slabQ[:, j, :], start=(j == 0), stop=(j == 7))
            if j == 7:
                nc.vector.tensor_scalar(
                    out=self.Q_T[:, pr, self.nb * TB:(self.nb + 1) * TB],
                    in0=self.acc[:], scalar1=self.qb_sb[:, pr:pr + 1],
                    scalar2=None, op0=ADD)
            self.idx += 1
            k -= 1


def _build():
    nc = bacc.Bacc("TRN2", target_bir_lowering=False, debug=False)
    xT16 = nc.dram_tensor("xT16", [1024, 2048], BF16, kind="ExternalInput").ap()
    wcat = nc.dram_tensor("wcat", [1024, 1536], BF16, kind="ExternalInput").ap()
    qb = nc.dram_tensor("qb", [128, 4], F32, kind="ExternalInput").ap()
    kb = nc.dram_tensor("kb", [128, 4], F32, kind="ExternalInput").ap()
    pw = nc.dram_tensor("pw", [512, 1024], BF16, kind="ExternalInput").ap()
    out = nc.dram_tensor("out", [2048, 1024], F32, kind="ExternalOutput").ap()
    scratch = nc.dram_tensor("scratch", [32, 512], F32).ap()
    scratch2 = nc.dram_tensor("scratch2", [32, 512], F32).ap()

    with tile.TileContext(nc) as tc, ExitStack() as ctx:
        sb = ctx.enter_context(tc.tile_pool(name="sb", bufs=1))
        ps = ctx.enter_context(tc.tile_pool(name="ps", bufs=1, space="PSUM"))

        w_sb = sb.tile([128, 8, 1536], BF16, tag="w")
        pw_sb = sb.tile([128, 4, 1024], BF16, tag="pw")
        Q_T = sb.tile([128, 4, 2048], BF16, tag="qt")
        K_T = sb.tile([128, 4, 2048], BF16, tag="kt")
        V_sb = sb.tile([128, 16, 8, 65], BF16, tag="v")
        qb_sb = sb.tile([128, 4], F32, tag="qb")
        kb_sb = sb.tile([128, 4], F32, tag="kb")
        zc = sb.tile([128, 8, 1], F32, tag="zc")
        onec = sb.tile([128, 1], F32, tag="onec")
        warm = sb.tile([128, 4], F32, tag="warm")

        # ---- initial DMAs, ordered so the first K matmul can start early.
        # slab/out on sync queue, weights on scalar queue, slab16/slabQ on
        # gpsimd queue.
        x_a = sb.tile([128, 8, 1024], BF16, tag="xa")
        x_b = sb.tile([128, 8, 1024], BF16, tag="xb")
        for j in range(8):
            nc.sync.dma_start(x_a[:, j, :],
                              xT16[j * 128:(j + 1) * 128, 0:1024])
        for j in range(8):  # full weight rows per chunk (3KB lines)
            nc.scalar.dma_start(w_sb[:, j, :], wcat[j * 128:(j + 1) * 128, :])
        for j in range(8):
            nc.sync.dma_start(x_b[:, j, :],
                              xT16[j * 128:(j + 1) * 128, 1024:2048])
        nc.scalar.dma_start(kb_sb[:], kb[:])
        nc.scalar.dma_start(qb_sb[:], qb[:])
        for pr in range(4):
            nc.scalar.dma_start(pw_sb[:, pr, :],
                                pw[pr * 128:(pr + 1) * 128, :])

        def x_tok(j, lo, n):
            if lo < 1024:
                return x_a[:, j, lo:lo + n]
            return x_b[:, j, lo - 1024:lo - 1024 + n]

        nc.vector.memset(zc[:], 0.0)
        nc.vector.memset(onec[:], 1.0)
        # preload the exp table set while phase A runs
        nc.scalar.activation(warm[0:1, 0:1], onec[0:1, 0:1], EXP,
                             bias=0.0, scale=1.0)
        for t in range(16):
            nc.vector.tensor_scalar(out=V_sb[:, t, :, 64:65], in0=zc[:],
                                    scalar1=onec[:], scalar2=None, op0=ADD)

        # ---- phase A: K and V for all tokens (4 blocks of 512)
        for nb in range(4):
            for pr in range(4):
                acc = ps.tile([128, TB], F32, tag="stage", bufs=2)
                for j in range(8):
                    nc.tensor.matmul(acc[:],
                                     w_sb[:, j, 512 + pr * 128:512 + (pr + 1) * 128],
                                     x_tok(j, nb * TB, TB),
                                     start=(j == 0), stop=(j == 7))
                nc.vector.tensor_scalar(out=K_T[:, pr, nb * TB:(nb + 1) * TB],
                                        in0=acc[:],
                                        scalar1=kb_sb[:, pr:pr + 1],
                                        scalar2=None, op0=ADD)
            for tc_i in range(4):
                t = nb * 4 + tc_i
                acc = ps.tile([128, TB], F32, tag="stage", bufs=2)
                for j in range(8):
                    nc.tensor.matmul(acc[:],
                                     x_tok(j, t * 128, 128),
                                     w_sb[:, j, 1024:1536],
                                     start=(j == 0), stop=(j == 7))
                nc.vector.tensor_copy(out=V_sb[:, t, :, 0:64],
                                      in_=acc[:].rearrange("p (h d) -> p h d", h=8))

        # ---- Q for all tokens
        for nb in range(4):
            for pr in range(4):
                acc = ps.tile([128, TB], F32, tag="stage", bufs=2)
                for j in range(8):
                    nc.tensor.matmul(acc[:],
                                     w_sb[:, j, pr * 128:(pr + 1) * 128],
                                     x_tok(j, nb * TB, TB),
                                     start=(j == 0), stop=(j == 7))
                nc.vector.tensor_scalar(out=Q_T[:, pr, nb * TB:(nb + 1) * TB],
                                        in0=acc[:],
                                        scalar1=qb_sb[:, pr:pr + 1],
                                        scalar2=None, op0=ADD)

        proj_blocks = []  # pending projection emission closures

        def make_proj_block(O_qb, qb_i, ns, co):
            def emit():
                pj = ps.tile([128, 512], F32, tag="pj", bufs=1)
                for pr in range(4):
                    nc.tensor.matmul(pj[:],
                                     O_qb[:, pr, ns * 128:(ns + 1) * 128],
                                     pw_sb[:, pr, co * 512:(co + 1) * 512],
                                     start=(pr == 0), stop=(pr == 3))
                so = sb.tile([128, 512], F32, tag="so", bufs=3)
                nc.vector.tensor_copy(out=so[:], in_=pj[:])
                nc.sync.dma_start(
                    out[qb_i * 512 + ns * 128:qb_i * 512 + (ns + 1) * 128,
                        co * 512:(co + 1) * 512], so[:])
            return emit

        norm_muls = []  # deferred normalize multiplies

        def emit_normalize(qb_i, pr, O_qb, oaug0, oaug1):
            # stage oaug into SBUF (frees the PSUM bank fast); denom row 64
            # -> reciprocal -> DRAM-bounce broadcast on the gpsimd queue.
            # The final multiply is deferred ~half a pr block so it never
            # blocks the strict-FIFO DVE queue waiting on the bounce DMA.
            for hh, oaug in ((0, oaug0), (1, oaug1)):
                row = qb_i * 8 + pr * 2 + hh
                ou = sb.tile([65, 512], F32, tag="ou", bufs=6)
                nc.vector.tensor_copy(out=ou[:], in_=oaug[:])
                # denominators -> DRAM -> [64, 8] (8/lane) -> reciprocal ->
                # DRAM -> broadcast read.  Keeps the DVE reciprocal wide
                # (250ns) instead of 3.3us single-lane.
                nc.sync.dma_start(scratch[row:row + 1, :], ou[64:65, :])
                d8 = sb.tile([64, 8], F32, tag="d8", bufs=4)
                nc.sync.dma_start(
                    d8[:], scratch[row:row + 1, :].rearrange(
                        "a (p f) -> (a p) f", p=64))
                r8 = sb.tile([64, 8], F32, tag="r8", bufs=4)
                nc.vector.reciprocal(r8[:], d8[:])
                nc.sync.dma_start(
                    scratch2[row:row + 1, :].rearrange(
                        "a (p f) -> (a p) f", p=64), r8[:])
                rb = sb.tile([64, 512], F32, tag="rb", bufs=4)
                nc.sync.dma_start(
                    rb[:], scratch2[row:row + 1, :].to_broadcast((64, 512)))

                def mul(hh=hh, ou=ou, rb=rb):
                    nc.vector.tensor_tensor(
                        out=O_qb[hh * 64:(hh + 1) * 64, pr, :],
                        in0=ou[0:64, :], in1=rb[:], op=MULT)
                norm_muls.append(mul)

        # ---- phase 2: one continuous software pipeline over (qb, pr, tg)
        # so the PE never sees a refill bubble (keeps HAM warm).  Each step:
        # PV of the previous step, scores of this step, exp of this step.
        steps = [(qb, pr, tg) for qb in range(4) for pr in range(4)
                 for tg in range(8)]
        O_qbs = {}
        ctx_of = {}   # (qb, pr) -> (oaug0, oaug1, O_qb)
        prev = None   # (P0, P1, t0, t1, qb, pr, tg)
        for qb_i, pr, tg in steps:
            q0 = qb_i * 512
            if tg == 0:
                if pr == 0:
                    O_qb_new = sb.tile([128, 4, 512], BF16, tag="oq", bufs=2)
                    O_qbs[qb_i] = O_qb_new
                oaug0_new = ps.tile([65, 512], F32, tag="oaug", bufs=3)
                oaug1_new = ps.tile([65, 512], F32, tag="oaug", bufs=3)
                ctx_of[(qb_i, pr)] = (oaug0_new, oaug1_new, O_qbs[qb_i])
            oaug0, oaug1, O_qb = ctx_of[(qb_i, pr)]
            # scores for this step first: exp(s) can start after 4 matmuls,
            # and the second t-group's weight load hides behind the other
            # row half
            t0, t1 = 2 * tg, 2 * tg + 1
            stage0 = ps.tile([128, 1024], F32, tag="stage", bufs=2)
            stage1 = ps.tile([128, 1024], F32, tag="stage", bufs=2)
            nc.tensor.matmul(stage0[:, 0:512],
                             K_T[0:64, pr, t0 * 128:(t0 + 1) * 128],
                             Q_T[0:64, pr, q0:q0 + 512],
                             start=True, stop=True, tile_position=(0, 0))
            nc.tensor.matmul(stage1[:, 0:512],
                             K_T[64:128, pr, t0 * 128:(t0 + 1) * 128],
                             Q_T[64:128, pr, q0:q0 + 512],
                             start=True, stop=True, tile_position=(64, 0))
            nc.tensor.matmul(stage0[:, 512:1024],
                             K_T[0:64, pr, t1 * 128:(t1 + 1) * 128],
                             Q_T[0:64, pr, q0:q0 + 512],
                             start=True, stop=True, tile_position=(0, 0))
            nc.tensor.matmul(stage1[:, 512:1024],
                             K_T[64:128, pr, t1 * 128:(t1 + 1) * 128],
                             Q_T[64:128, pr, q0:q0 + 512],
                             start=True, stop=True, tile_position=(64, 0))
            # PV for the previous step (lags one step, crosses boundaries)
            if prev is not None:
                pP0, pP1, pt0, pt1, pqb, ppr, ptg = prev
                po0, po1, pO = ctx_of[(pqb, ppr)]
                st, sp = (ptg == 0), (ptg == 7)
                nc.tensor.matmul(po0[:], V_sb[:, pt0, 2 * ppr, :],
                                 pP0[:, 0:512], start=st, stop=False)
                nc.tensor.matmul(po0[:], V_sb[:, pt1, 2 * ppr, :],
                                 pP0[:, 512:1024], start=False, stop=sp)
                nc.tensor.matmul(po1[:], V_sb[:, pt0, 2 * ppr + 1, :],
                                 pP1[:, 0:512], start=st, stop=False)
                nc.tensor.matmul(po1[:], V_sb[:, pt1, 2 * ppr + 1, :],
                                 pP1[:, 512:1024], start=False, stop=sp)
                if sp:
                    emit_normalize(pqb, ppr, pO, po0, po1)
                    if ppr == 3:
                        for ns in range(4):
                            for co in range(2):
                                proj_blocks.append(
                                    make_proj_block(pO, pqb, ns, co))
            # deferred background work: normalize multiplies (half a pr
            # later), then projection blocks of the previous query block
            if tg == 4 and norm_muls:
                norm_muls.pop(0)()
                if norm_muls:
                    norm_muls.pop(0)()
            if pr in (1, 2) and tg % 2 == 1 and proj_blocks:
                proj_blocks.pop(0)()
            # exp for this step
            P0 = sb.tile([128, 1024], BF16, tag="p", bufs=6)
            P1 = sb.tile([128, 1024], BF16, tag="p", bufs=6)
            nc.scalar.activation(P0[:], stage0[:], EXP, bias=0.0, scale=SCALE)
            nc.scalar.activation(P1[:], stage1[:], EXP, bias=0.0, scale=SCALE)
            prev = (P0, P1, t0, t1, qb_i, pr, tg)
        # drain: final PV, final normalize, remaining projections
        pP0, pP1, pt0, pt1, pqb, ppr, ptg = prev
        po0, po1, pO = ctx_of[(pqb, ppr)]
        nc.tensor.matmul(po0[:], V_sb[:, pt0, 2 * ppr, :],
                         pP0[:, 0:512], start=False, stop=False)
        nc.tensor.matmul(po0[:], V_sb[:, pt1, 2 * ppr, :],
                         pP0[:, 512:1024], start=False, stop=True)
        nc.tensor.matmul(po1[:], V_sb[:, pt0, 2 * ppr + 1, :],
                         pP1[:, 0:512], start=False, stop=False)
        nc.tensor.matmul(po1[:], V_sb[:, pt1, 2 * ppr + 1, :],
                         pP1[:, 512:1024], start=False, stop=True)
        emit_normalize(pqb, ppr, pO, po0, po1)
        for _ in range(14):
            dummy = ps.tile([128, 512], F32, tag="stage", bufs=2)
            for jj in range(4):
                nc.tensor.matmul(dummy[:], K_T[0:64, 0, 0:128],
                                 Q_T[0:64, 0, 0:512], start=(jj == 0),
                                 stop=(jj == 3))
        while norm_muls:
            norm_muls.pop(0)()
        for ns in range(4):
            for co in range(2):
                proj_blocks.append(make_proj_block(pO, pqb, ns, co))
        while proj_blocks:
            proj_blocks.pop(0)()
    return nc


def _prepare_in_maps(x, qkv_w, qkv_b, proj_w):
    x = np.asarray(x, dtype=np.float32)
    w = np.asarray(qkv_w, dtype=np.float32)
    pwr = np.asarray(proj_w, dtype=np.float32)
    qkv_b = np.asarray(qkv_b, dtype=np.float32)
    in_maps = []
    for c in range(8):
        b, g = c % 4, c // 4
        w0 = 512 * g
        xt = np.ascontiguousarray(x[b].T)
        in_maps.append({
            "xT16": _bf16(xt),
            "wcat": _bf16(np.concatenate(
                [w[:, w0:w0 + 512],
                 w[:, 1024 + w0:1024 + w0 + 512],
                 w[:, 2048 + w0:2048 + w0 + 512]], axis=1)),
            "qb": np.ascontiguousarray(qkv_b[w0:w0 + 512].reshape(4, 128).T),
            "kb": np.ascontiguousarray(
                qkv_b[1024 + w0:1024 + w0 + 512].reshape(4, 128).T),
            "pw": _bf16(pwr[w0:w0 + 512, :]),
        })
    return in_maps


def _gather(parts, qkv_b, proj_w, proj_b):
    const_row = (np.asarray(qkv_b)[2048:].astype(np.float64)
                 @ np.asarray(proj_w).astype(np.float64)
                 + np.asarray(proj_b).astype(np.float64))
    out = np.empty((B, N, C), np.float32)
    for b in range(B):
        out[b] = (parts[b].astype(np.float64) + parts[b + 4].astype(np.float64)
                  + const_row).astype(np.float32)
    return out


def kernel(**inputs: np.ndarray) -> np.ndarray:
    x = np.asarray(inputs["x"], dtype=np.float32)
    qkv_w = np.asarray(inputs["qkv_w"], dtype=np.float32)
    qkv_b = np.asarray(inputs["qkv_b"], dtype=np.float32)
    proj_w = np.asarray(inputs["proj_w"], dtype=np.float32)
    proj_b = np.asarray(inputs["proj_b"], dtype=np.float32)

    in_maps = _prepare_in_maps(x, qkv_w, qkv_b, proj_w)
    nc = _build()
    nc.finalize()
    res = run_bass_kernel_spmd(nc, in_maps, list(range(8)))
    parts = [res.results[c]["out"] for c in range(8)]
    return _gather(parts, qkv_b, proj_w, proj_b)


if __name__ == "__main__":
    import tempfile
    import time

    from concourse.bass_utils import compile_bass_kernel

    t0 = time.time()
    nc = _build()
    nc.compile()
    with tempfile.TemporaryDirectory() as td:
        compile_bass_kernel(nc, td, neff_name="k.neff")
    print(f"COMPILE OK ({time.time() - t0:.0f}s)", flush=True)
